# revision 2
# baseline (speedup 1.0000x reference)
"""Trainium2 Bass kernel for the intra-batch point-cloud contrastive loss.

Math (matches the reference exactly):
  feats   = features_in.reshape(C, M).T    (row-major reinterpret), M = B*N
  labels  = labels_in.reshape(-1)
  sel     = bernoulli(key 42, min(750/(count+1),1)[labels])   (host, jax CPU)
  nv      = feats / ||feats||
  dp      = exp(nv @ nv.T / TEMP), diagonal zeroed
  pos_i   = sum_{j sel, same class} dp_ij ; neg over different class
  loss    = mean over selected i of -log(pos/(pos+neg))

Only selected points contribute (unselected rows give 0 loss, unselected
columns have zero weight), so the device works on the compacted point set
(~37% of M).  Columns are sharded over 8 cores.  To keep the program
SPMD-identical, each core receives the compacted feature matrix *rolled*
so its own columns sit first; the diagonal then lands at a fixed position
for every core and is zeroed with one shared (1-eye) mask.

Per core (L = M_pad/8 local columns, nT = M_pad/128 row chunks):
  mm1 (PE):  G_t = nvT[:, chunk t].T @ nvT[:, :L]     [128, L] fp32 psum
  exp (ACT): dp_t = exp(G_t / TEMP)                    -> SBUF
  mask(DVE): zero the diagonal sub-block (t < L/128)
  mm2 (PE):  S += W_t.T @ dp_t   (W = sel*onehot(label), [4, L] psum accum)
The host gathers the per-core S blocks and finishes the O(n_sel) epilogue.
"""

import numpy as np

TEMP = 0.07
NUM_CLASSES = 4
N_CORES = 8
P = 128

_NEFF_CACHE = {}


def _compute_sel(labels_flat):
    """Selection mask, bit-exact with the reference (jax threefry, key 42)."""
    import jax
    import jax.numpy as jnp

    cpu = jax.devices("cpu")[0]
    with jax.default_device(cpu):
        lab_j = jnp.asarray(labels_flat)
        counts = jnp.bincount(lab_j, length=NUM_CLASSES)
        keep_p = jnp.minimum(750.0 / (counts.astype(jnp.float32) + 1.0), 1.0)
        p = keep_p[lab_j]
        sel = jax.random.bernoulli(jax.random.key(42), p)
        return np.asarray(sel)


def _build_kernel(M_pad):
    import concourse.bass as bass
    import concourse.mybir as mybir
    import concourse.tile as tile

    L = M_pad // N_CORES          # local columns per core
    nT = M_pad // P               # 128-row chunks
    nL = L // P                   # chunks containing this core's diagonal
    f32 = mybir.dt.float32

    # consts layout (single tensor -> single DMA -> single semaphore):
    # cols [0, nT*4)                     W chunks (mm2 lhsT)
    # cols [nT*4, nT*4+P)                128x128 identity
    # cols [nT*4+P, nT*4+P+2L-P)         dwide (-1e9 shifted diagonal)
    CW = nT * NUM_CLASSES + P + (2 * L - P)
    o_eye = nT * NUM_CLASSES
    o_dw = o_eye + P

    nc = bass.Bass()
    packed_d = nc.dram_tensor("packed", [P, M_pad + CW], f32, kind="ExternalInput")
    s_d = nc.dram_tensor("s_out", [NUM_CLASSES, L], f32, kind="ExternalOutput")

    with tile.TileContext(nc) as tc:
        with (
            tc.tile_pool(name="singles", bufs=1) as singles,
            tc.tile_pool(name="dp", bufs=nT) as dp_pool,
            tc.tile_pool(name="ps", bufs=7, space="PSUM") as ps_pool,
            tc.tile_pool(name="acc", bufs=1, space="PSUM") as acc_pool,
        ):
            packed = singles.tile([P, M_pad + CW], f32)
            # ONE SWDGE (gpsimd) DMA -> one completion semaphore.  Several
            # DMAs (or an HWDGE multi-queue fan-out) would attach more inline
            # sync waits than this walrus build allows per instruction.
            nc.gpsimd.dma_start(out=packed[:], in_=packed_d[:])
            nvt = packed[0:64, 0:M_pad]
            w_sb = packed[:, M_pad + 0:M_pad + o_eye]
            eye_sb = packed[:, M_pad + o_eye:M_pad + o_dw]
            dwide_sb = packed[:, M_pad + o_dw:M_pad + CW]

            s_ps = acc_pool.tile([NUM_CLASSES, L], f32)
            rhs = nvt[:, 0:L]
            # dwide[p, c] = -1e9 iff c == p + (nL-1)*P; sliced so the -1e9
            # diagonal lands on this chunk's own columns [t*P, t*P+P).
            off0 = (nL - 1) * P
            for t in range(nT):
                ps = ps_pool.tile([P, L], f32)
                nc.tensor.matmul(
                    ps[:], nvt[:, t * P:(t + 1) * P], rhs,
                    start=True, stop=(t >= nL),
                )
                if t < nL:
                    # G += I.T @ D = D: pushes the diagonal to -1e9 so that
                    # exp() maps it to exactly 0.
                    nc.tensor.matmul(
                        ps[:], eye_sb, dwide_sb[:, off0 - t * P: off0 - t * P + L],
                        start=False, stop=True,
                    )
                dp = dp_pool.tile([P, L], f32)
                nc.scalar.activation(
                    dp[:], ps[:], mybir.ActivationFunctionType.Exp,
                    scale=float(1.0 / TEMP),
                )
                nc.tensor.matmul(
                    s_ps[:], w_sb[:, t * NUM_CLASSES:(t + 1) * NUM_CLASSES], dp[:],
                    start=(t == 0), stop=(t == nT - 1),
                )

            s_sb = singles.tile([NUM_CLASSES, L], f32)
            nc.scalar.copy(s_sb[:], s_ps[:])
            nc.gpsimd.dma_start(out=s_d[:], in_=s_sb[:])

    _split_multi_waits(nc)
    return nc


def _split_multi_waits(nc):
    """Walrus in this toolchain accepts only one inline sync-wait per
    instruction.  Tile's kernel-tail drain aggregates one wait per live
    semaphore, so hoist all but the last wait onto same-engine nops."""
    import concourse.mybir as mybir

    for fn in nc.m.functions:
        for blk in fn.blocks:
            insts = list(blk.instructions)
            out = []
            for inst in insts:
                si = inst.sync_info
                waits = list(si.on_wait) if si is not None and si.on_wait else []
                if len(waits) > 1:
                    for w in waits[:-1]:
                        out.append(mybir.InstNoOp(
                            name=nc.get_next_instruction_name(),
                            engine=inst.engine,
                            bass_nofuse=True,
                            sync_info=mybir.SyncInfo(on_wait=[w], on_update=[]),
                        ))
                    si.on_wait = waits[-1:]
                out.append(inst)
            if len(out) != len(insts):
                blk.instructions = out


def _get_kernel(M_pad):
    if M_pad not in _NEFF_CACHE:
        _NEFF_CACHE[M_pad] = _build_kernel(M_pad)
    return _NEFF_CACHE[M_pad]


_results = [None]


def kernel(features_in, labels_in, _trace=False, _results=_results):
    from concourse.bass_utils import run_bass_kernel_spmd

    features_in = np.asarray(features_in, dtype=np.float32)
    B, C, N = features_in.shape
    M = B * N
    labels = np.asarray(labels_in).reshape(-1).astype(np.int64)

    fT = features_in.reshape(C, M)                      # [C, M] reinterpret
    sel = _compute_sel(labels)
    idx = np.nonzero(sel)[0]
    n_sel = int(idx.size)
    n_div = max(n_sel, 1)

    norms = np.sqrt(np.sum(fT * fT, axis=0, dtype=np.float32)).astype(np.float32)
    nvT = (fT / norms).astype(np.float32)

    lab_sel = labels[idx]
    per_core = N_CORES * P
    M_pad = max(((n_sel + per_core - 1) // per_core) * per_core, per_core)
    L = M_pad // N_CORES
    nT = M_pad // P

    nvT_pad = np.zeros((C, M_pad), np.float32)
    nvT_pad[:, :n_sel] = nvT[:, idx]
    W = np.zeros((M_pad, NUM_CLASSES), np.float32)
    W[np.arange(n_sel), lab_sel] = 1.0

    nL = L // P
    eye = np.eye(P, dtype=np.float32)
    dwide = np.zeros((P, 2 * L - P), np.float32)
    dwide[np.arange(P), np.arange(P) + (nL - 1) * P] = -1e9

    in_maps = []
    for k in range(N_CORES):
        nv_k = np.ascontiguousarray(np.roll(nvT_pad, -L * k, axis=1))
        W_k = np.roll(W, -L * k, axis=0)
        # lhsT chunk t lives at columns [4t, 4t+4): w_arr[p, 4t+c] = W_k[128t+p, c]
        w_arr = W_k.reshape(nT, P, NUM_CLASSES).transpose(1, 0, 2).reshape(
            P, nT * NUM_CLASSES
        )
        consts = np.concatenate([w_arr, eye, dwide], axis=1)
        packed = np.zeros((P, M_pad + consts.shape[1]), np.float32)
        packed[:C, :M_pad] = nv_k
        packed[:, M_pad:] = consts
        in_maps.append({"packed": packed})

    nc = _get_kernel(M_pad)
    res = run_bass_kernel_spmd(nc, in_maps, core_ids=list(range(N_CORES)),
                               trace=_trace)
    _results[0] = res

    S = np.concatenate([res.results[k]["s_out"] for k in range(N_CORES)], axis=1)
    S = S[:, :n_sel]
    denom = np.sum(S, axis=0, dtype=np.float32).astype(np.float32)
    numer = S[lab_sel, np.arange(n_sel)]
    per = (-np.log(numer / denom)).astype(np.float32)
    loss = np.float32(per.sum(dtype=np.float32) / np.float32(n_div))
    return np.asarray(loss, dtype=np.float32)



# revision 4
# speedup vs baseline: 1.8464x; 1.8464x over previous
"""Trainium2 Bass kernel for the intra-batch point-cloud contrastive loss.

Math (matches the reference exactly):
  feats   = features_in.reshape(C, M).T    (row-major reinterpret), M = B*N
  labels  = labels_in.reshape(-1)
  sel     = bernoulli(key 42, min(750/(count+1),1)[labels])   (host, jax CPU)
  nv      = feats / ||feats||
  dp      = exp(nv @ nv.T / TEMP), diagonal zeroed
  pos_i   = sum_{j sel, same class} dp_ij ; neg over different class
  loss    = mean over selected i of -log(pos/(pos+neg))

Only selected points contribute, so the device works on the compacted
point set (~37% of M).  Columns are sharded over 8 cores; each core
receives the compacted feature matrix *rolled* so its own columns sit
first, which pins the sim diagonal at a fixed (compile-time) position
for every core (SPMD-safe).

Per core (L = M_pad/8 local columns, nT = M_pad/128 row chunks,
grouped GS=3 chunks per PSUM tile):
  mm1 (PE):   G_t = nvT[:, chunk t].T @ nvT[:, :L]  fp16 -> [128, L] fp32,
              written into 512-aligned slots of a [128, 3*512] psum tile
  exp (ACT):  one wide strided ACTIVATE per group: dp = exp(G/TEMP),
              [128, 3, L] psum -> dense [128, 3*L] bf16 SBUF
              (the (N+352)cyc ACTIVATE overhead amortizes 3x vs per-chunk)
  diag (DVE): affine_select zeroes the 3 diagonal 128-blocks (group 0)
  mm2 (PE):   S += W_t.T @ dp_t  (W = sel*onehot(label) bf16, [4, L] psum)
The host gathers the per-core S blocks and finishes the O(n_sel) epilogue.
"""

import numpy as np

TEMP = 0.07
NUM_CLASSES = 4
N_CORES = 8
P = 128
GS = 3            # row chunks per psum tile / per ACTIVATE
SLOT = 512        # psum slot pitch (fp32 bank = 512 cols)

_NEFF_CACHE = {}
_results = [None]


def _compute_sel(labels_flat):
    """Selection mask, bit-exact with the reference (jax threefry, key 42)."""
    import jax
    import jax.numpy as jnp

    cpu = jax.devices("cpu")[0]
    with jax.default_device(cpu):
        lab_j = jnp.asarray(labels_flat)
        counts = jnp.bincount(lab_j, length=NUM_CLASSES)
        keep_p = jnp.minimum(750.0 / (counts.astype(jnp.float32) + 1.0), 1.0)
        p = keep_p[lab_j]
        sel = jax.random.bernoulli(jax.random.key(42), p)
        return np.asarray(sel)


def _build_kernel(M_pad):
    import concourse.bass as bass
    import concourse.mybir as mybir
    import concourse.tile as tile

    L = M_pad // N_CORES          # local columns per core
    nT = M_pad // P               # 128-row chunks
    nL = L // P                   # chunks containing this core's diagonal
    f32 = mybir.dt.float32
    f16 = mybir.dt.float16
    bf16 = mybir.dt.bfloat16
    assert nL <= GS, "diagonal chunks must fit in the first group"

    nc = bass.Bass()
    nvt_d = nc.dram_tensor("nvt", [64, M_pad], f16, kind="ExternalInput")
    w_d = nc.dram_tensor("w", [P, nT * NUM_CLASSES], bf16, kind="ExternalInput")
    s_d = nc.dram_tensor("s_out", [NUM_CLASSES, L], f32, kind="ExternalOutput")

    groups = [(g, min(GS, nT - g)) for g in range(0, nT, GS)]

    with tile.TileContext(nc) as tc:
        with (
            tc.tile_pool(name="singles", bufs=1) as singles,
            tc.tile_pool(name="dp", bufs=2) as dp_pool,
            tc.tile_pool(name="ps", bufs=2, space="PSUM") as ps_pool,
            tc.tile_pool(name="acc", bufs=1, space="PSUM") as acc_pool,
        ):
            nvt = singles.tile([64, M_pad], f16)
            w_sb = singles.tile([P, nT * NUM_CLASSES], bf16)
            nc.gpsimd.dma_start(out=nvt[:], in_=nvt_d[:])
            nc.gpsimd.dma_start(out=w_sb[:], in_=w_d[:])

            s_ps = acc_pool.tile([NUM_CLASSES, L], f32)
            rhs = nvt[:, 0:L]
            prev = None          # deferred mm2 work: (dp tile, g0, n)
            for g0, n in groups:
                ps = ps_pool.tile([P, GS * SLOT], f32)
                for i in range(n):
                    t = g0 + i
                    nc.tensor.matmul(
                        ps[:, i * SLOT:i * SLOT + L],
                        nvt[:, t * P:(t + 1) * P], rhs,
                        start=True, stop=True,
                    )
                dp = dp_pool.tile([P, GS * L], bf16)
                src = ps[:, 0:n * SLOT].rearrange("p (g c) -> p g c", c=SLOT)
                src = src[:, :, 0:L]
                dst = dp[:, 0:n * L].rearrange("p (g c) -> p g c", c=L)
                nc.scalar.activation(
                    dst, src, mybir.ActivationFunctionType.Exp,
                    scale=float(1.0 / TEMP),
                )
                if g0 == 0:
                    # zero the diagonal: chunk t's diagonal element sits at
                    # dp[p, t*L + t*P + p]; keep where (col - p) != 0.
                    for t in range(nL):
                        blk = dp[:, t * L + t * P: t * L + t * P + P]
                        nc.gpsimd.affine_select(
                            out=blk, in_=blk,
                            compare_op=mybir.AluOpType.not_equal,
                            fill=0.0, base=0,
                            pattern=[[1, P]], channel_multiplier=-1,
                        )
                if prev is not None:
                    pdp, pg0, pn = prev
                    for i in range(pn):
                        t = pg0 + i
                        nc.tensor.matmul(
                            s_ps[:], w_sb[:, t * NUM_CLASSES:(t + 1) * NUM_CLASSES],
                            pdp[:, i * L:(i + 1) * L],
                            start=(t == 0), stop=False,
                        )
                prev = (dp, g0, n)
            pdp, pg0, pn = prev
            for i in range(pn):
                t = pg0 + i
                nc.tensor.matmul(
                    s_ps[:], w_sb[:, t * NUM_CLASSES:(t + 1) * NUM_CLASSES],
                    pdp[:, i * L:(i + 1) * L],
                    start=(t == 0), stop=(t == nT - 1),
                )

            s_sb = singles.tile([NUM_CLASSES, L], f32)
            nc.vector.tensor_copy(s_sb[:], s_ps[:])
            nc.gpsimd.dma_start(out=s_d[:], in_=s_sb[:])

    _split_multi_waits(nc)
    return nc


def _split_multi_waits(nc):
    """Walrus in this toolchain accepts only one inline sync-wait per
    instruction.  Tile's kernel-tail drain aggregates one wait per live
    semaphore, so hoist all but the last wait onto same-engine nops."""
    import concourse.mybir as mybir

    for fn in nc.m.functions:
        for blk in fn.blocks:
            insts = list(blk.instructions)
            out = []
            for inst in insts:
                si = inst.sync_info
                waits = list(si.on_wait) if si is not None and si.on_wait else []
                if len(waits) > 1:
                    for w in waits[:-1]:
                        out.append(mybir.InstNoOp(
                            name=nc.get_next_instruction_name(),
                            engine=inst.engine,
                            bass_nofuse=True,
                            sync_info=mybir.SyncInfo(on_wait=[w], on_update=[]),
                        ))
                    si.on_wait = waits[-1:]
                out.append(inst)
            if len(out) != len(insts):
                blk.instructions = out


def _get_kernel(M_pad):
    if M_pad not in _NEFF_CACHE:
        _NEFF_CACHE[M_pad] = _build_kernel(M_pad)
    return _NEFF_CACHE[M_pad]


def kernel(features_in, labels_in, _trace=False, _results=_results):
    import ml_dtypes
    from concourse.bass_utils import run_bass_kernel_spmd

    features_in = np.asarray(features_in, dtype=np.float32)
    B, C, N = features_in.shape
    M = B * N
    labels = np.asarray(labels_in).reshape(-1).astype(np.int64)

    fT = features_in.reshape(C, M)                      # [C, M] reinterpret
    sel = _compute_sel(labels)
    idx = np.nonzero(sel)[0]
    n_sel = int(idx.size)
    n_div = max(n_sel, 1)

    norms = np.sqrt(np.sum(fT * fT, axis=0, dtype=np.float32)).astype(np.float32)
    nvT = (fT / norms).astype(np.float32)

    lab_sel = labels[idx]
    per_core = N_CORES * P
    M_pad = max(((n_sel + per_core - 1) // per_core) * per_core, per_core)
    L = M_pad // N_CORES
    nT = M_pad // P

    nvT_pad = np.zeros((C, M_pad), np.float16)
    nvT_pad[:, :n_sel] = nvT[:, idx].astype(np.float16)
    W = np.zeros((M_pad, NUM_CLASSES), np.float32)
    W[np.arange(n_sel), lab_sel] = 1.0

    in_maps = []
    for k in range(N_CORES):
        nv_k = np.ascontiguousarray(np.roll(nvT_pad, -L * k, axis=1))
        W_k = np.roll(W, -L * k, axis=0)
        # lhsT chunk t lives at columns [4t, 4t+4): w_arr[p, 4t+c] = W_k[128t+p, c]
        w_arr = W_k.reshape(nT, P, NUM_CLASSES).transpose(1, 0, 2).reshape(
            P, nT * NUM_CLASSES
        ).astype(ml_dtypes.bfloat16)
        in_maps.append({"nvt": nv_k, "w": w_arr})

    nc = _get_kernel(M_pad)
    res = run_bass_kernel_spmd(nc, in_maps, core_ids=list(range(N_CORES)),
                               trace=_trace)
    _results[0] = res

    S = np.concatenate([res.results[k]["s_out"] for k in range(N_CORES)], axis=1)
    S = S[:, :n_sel].astype(np.float64)
    denom = np.sum(S, axis=0)
    numer = S[lab_sel, np.arange(n_sel)]
    per = -np.log(numer / denom)
    loss = np.float32(per.sum() / n_div)
    return np.asarray(loss, dtype=np.float32)


# revision 9
# speedup vs baseline: 1.9503x; 1.0562x over previous
"""Trainium2 Bass kernel for the intra-batch point-cloud contrastive loss.

Math (matches the reference exactly):
  feats   = features_in.reshape(C, M).T    (row-major reinterpret), M = B*N
  labels  = labels_in.reshape(-1)
  sel     = bernoulli(key 42, min(750/(count+1),1)[labels])   (host, jax CPU)
  nv      = feats / ||feats||
  dp      = exp(nv @ nv.T / TEMP), diagonal zeroed
  pos_i   = sum_{j sel, same class} dp_ij ; neg over different class
  loss    = mean over selected i of -log(pos/(pos+neg))

Only selected points contribute, so the device works on the compacted
point set (~37% of M).  Columns are sharded over 8 cores; each core
receives the compacted feature matrix *rolled* so its own columns sit
first, which pins the sim diagonal at a fixed (compile-time) position
for every core (SPMD-safe).

Per core (L = M_pad/8 local columns, nT = M_pad/128 row chunks,
grouped GS=3 chunks per PSUM tile):
  mm1 (PE):   G_t = nvT[:, chunk t].T @ nvT[:, :L]  fp16 -> [128, L] fp32,
              written into 512-aligned slots of a [128, 3*512] psum tile
  exp (ACT):  one wide strided ACTIVATE per group: dp = exp(G/TEMP),
              [128, 3, L] psum -> dense [128, 3*L] bf16 SBUF
              (the (N+352)cyc ACTIVATE overhead amortizes 3x vs per-chunk)
  diag (DVE): affine_select zeroes the 3 diagonal 128-blocks (group 0)
  mm2 (PE):   S += W_t.T @ dp_t  (W = sel*onehot(label) bf16, [4, L] psum)
The host gathers the per-core S blocks and finishes the O(n_sel) epilogue.
"""

import numpy as np

TEMP = 0.07
NUM_CLASSES = 4
N_CORES = 8
P = 128
GS = 2            # row chunks per psum tile / per ACTIVATE
SLOT = 512        # psum slot pitch (fp32 bank = 512 cols)

_NEFF_CACHE = {}
_results = [None]


def _compute_sel(labels_flat):
    """Selection mask, bit-exact with the reference (jax threefry, key 42)."""
    import jax
    import jax.numpy as jnp

    cpu = jax.devices("cpu")[0]
    with jax.default_device(cpu):
        lab_j = jnp.asarray(labels_flat)
        counts = jnp.bincount(lab_j, length=NUM_CLASSES)
        keep_p = jnp.minimum(750.0 / (counts.astype(jnp.float32) + 1.0), 1.0)
        p = keep_p[lab_j]
        sel = jax.random.bernoulli(jax.random.key(42), p)
        return np.asarray(sel)


def _build_kernel(M_pad):
    import concourse.bass as bass
    import concourse.mybir as mybir
    import concourse.tile as tile

    L = M_pad // N_CORES          # local columns per core
    nT = M_pad // P               # 128-row chunks
    nL = L // P                   # chunks containing this core's diagonal
    f32 = mybir.dt.float32
    f16 = mybir.dt.float16
    bf16 = mybir.dt.bfloat16

    nc = bass.Bass()
    nvt_d = nc.dram_tensor("nvt", [64, M_pad], f16, kind="ExternalInput")
    w_d = nc.dram_tensor("w", [P, nT * NUM_CLASSES], bf16, kind="ExternalInput")
    s_d = nc.dram_tensor("s_out", [NUM_CLASSES, L], f32, kind="ExternalOutput")

    groups = [(g, min(GS, nT - g)) for g in range(0, nT, GS)]
    split = min(8 * P, M_pad)     # first DMA piece: rhs + early lhsT chunks

    with tile.TileContext(nc) as tc:
        with (
            tc.tile_pool(name="singles", bufs=1) as singles,
            tc.tile_pool(name="dp", bufs=3) as dp_pool,
            tc.tile_pool(name="ps", bufs=3, space="PSUM") as ps_pool,
            tc.tile_pool(name="acc", bufs=1, space="PSUM") as acc_pool,
        ):
            nvt = singles.tile([64, M_pad], f16)
            w_sb = singles.tile([P, nT * NUM_CLASSES], bf16)
            nc.gpsimd.dma_start(out=nvt[:, 0:split], in_=nvt_d[:, 0:split])
            if split < M_pad:
                nc.gpsimd.dma_start(out=nvt[:, split:M_pad],
                                    in_=nvt_d[:, split:M_pad])
            nc.gpsimd.dma_start(out=w_sb[:], in_=w_d[:])

            s_ps = acc_pool.tile([NUM_CLASSES, L], f32)
            rhs = nvt[:, 0:L]
            prev = None          # deferred mm2 work: (dp tile, g0, n)
            for g0, n in groups:
                ps = ps_pool.tile([P, GS * SLOT], f32)
                for i in range(n):
                    t = g0 + i
                    nc.tensor.matmul(
                        ps[:, i * SLOT:i * SLOT + L],
                        nvt[:, t * P:(t + 1) * P], rhs,
                        start=True, stop=True,
                    )
                dp = dp_pool.tile([P, GS * L], bf16)
                src = ps[:, 0:n * SLOT].rearrange("p (g c) -> p g c", c=SLOT)
                src = src[:, :, 0:L]
                dst = dp[:, 0:n * L].rearrange("p (g c) -> p g c", c=L)
                nc.scalar.activation(
                    dst, src, mybir.ActivationFunctionType.Exp,
                    scale=float(1.0 / TEMP),
                )
                for i in range(n):
                    # zero the diagonal: chunk t's diagonal element sits at
                    # dp[p, i*L + t*P + p]; keep where (col - p) != 0.
                    t = g0 + i
                    if t >= nL:
                        break
                    blk = dp[:, i * L + t * P: i * L + t * P + P]
                    nc.gpsimd.affine_select(
                        out=blk, in_=blk,
                        compare_op=mybir.AluOpType.not_equal,
                        fill=0.0, base=0,
                        pattern=[[1, P]], channel_multiplier=-1,
                    )
                if prev is not None:
                    pdp, pg0, pn = prev
                    for i in range(pn):
                        t = pg0 + i
                        nc.tensor.matmul(
                            s_ps[:], w_sb[:, t * NUM_CLASSES:(t + 1) * NUM_CLASSES],
                            pdp[:, i * L:(i + 1) * L],
                            start=(t == 0), stop=False,
                        )
                prev = (dp, g0, n)
            pdp, pg0, pn = prev
            for i in range(pn):
                t = pg0 + i
                nc.tensor.matmul(
                    s_ps[:], w_sb[:, t * NUM_CLASSES:(t + 1) * NUM_CLASSES],
                    pdp[:, i * L:(i + 1) * L],
                    start=(t == 0), stop=(t == nT - 1),
                )

            s_sb = singles.tile([NUM_CLASSES, L], f32)
            nc.vector.tensor_copy(s_sb[:], s_ps[:])
            nc.gpsimd.dma_start(out=s_d[:], in_=s_sb[:])

    _split_multi_waits(nc)
    return nc


def _split_multi_waits(nc):
    """Walrus in this toolchain accepts only one inline sync-wait per
    instruction.  Tile's kernel-tail drain aggregates one wait per live
    semaphore, so hoist all but the last wait onto same-engine nops."""
    import concourse.mybir as mybir

    for fn in nc.m.functions:
        for blk in fn.blocks:
            insts = list(blk.instructions)
            out = []
            for inst in insts:
                si = inst.sync_info
                waits = list(si.on_wait) if si is not None and si.on_wait else []
                if len(waits) > 1:
                    for w in waits[:-1]:
                        out.append(mybir.InstNoOp(
                            name=nc.get_next_instruction_name(),
                            engine=inst.engine,
                            bass_nofuse=True,
                            sync_info=mybir.SyncInfo(on_wait=[w], on_update=[]),
                        ))
                    si.on_wait = waits[-1:]
                out.append(inst)
            if len(out) != len(insts):
                blk.instructions = out


def _get_kernel(M_pad):
    if M_pad not in _NEFF_CACHE:
        _NEFF_CACHE[M_pad] = _build_kernel(M_pad)
    return _NEFF_CACHE[M_pad]


def kernel(features_in, labels_in, _trace=False, _results=_results):
    import ml_dtypes
    from concourse.bass_utils import run_bass_kernel_spmd

    features_in = np.asarray(features_in, dtype=np.float32)
    B, C, N = features_in.shape
    M = B * N
    labels = np.asarray(labels_in).reshape(-1).astype(np.int64)

    fT = features_in.reshape(C, M)                      # [C, M] reinterpret
    sel = _compute_sel(labels)
    idx = np.nonzero(sel)[0]
    n_sel = int(idx.size)
    n_div = max(n_sel, 1)

    norms = np.sqrt(np.sum(fT * fT, axis=0, dtype=np.float32)).astype(np.float32)
    nvT = (fT / norms).astype(np.float32)

    lab_sel = labels[idx]
    per_core = N_CORES * P
    M_pad = max(((n_sel + per_core - 1) // per_core) * per_core, per_core)
    L = M_pad // N_CORES
    nT = M_pad // P

    nvT_pad = np.zeros((C, M_pad), np.float16)
    nvT_pad[:, :n_sel] = nvT[:, idx].astype(np.float16)
    W = np.zeros((M_pad, NUM_CLASSES), np.float32)
    W[np.arange(n_sel), lab_sel] = 1.0

    in_maps = []
    for k in range(N_CORES):
        nv_k = np.ascontiguousarray(np.roll(nvT_pad, -L * k, axis=1))
        W_k = np.roll(W, -L * k, axis=0)
        # lhsT chunk t lives at columns [4t, 4t+4): w_arr[p, 4t+c] = W_k[128t+p, c]
        w_arr = W_k.reshape(nT, P, NUM_CLASSES).transpose(1, 0, 2).reshape(
            P, nT * NUM_CLASSES
        ).astype(ml_dtypes.bfloat16)
        in_maps.append({"nvt": nv_k, "w": w_arr})

    nc = _get_kernel(M_pad)
    res = run_bass_kernel_spmd(nc, in_maps, core_ids=list(range(N_CORES)),
                               trace=_trace)
    _results[0] = res

    S = np.concatenate([res.results[k]["s_out"] for k in range(N_CORES)], axis=1)
    S = S[:, :n_sel].astype(np.float64)
    denom = np.sum(S, axis=0)
    numer = S[lab_sel, np.arange(n_sel)]
    per = -np.log(numer / denom)
    loss = np.float32(per.sum() / n_div)
    return np.asarray(loss, dtype=np.float32)


# revision 11
# speedup vs baseline: 1.9553x; 1.0026x over previous
"""Trainium2 Bass kernel for the intra-batch point-cloud contrastive loss.

Math (matches the reference exactly):
  feats   = features_in.reshape(C, M).T    (row-major reinterpret), M = B*N
  labels  = labels_in.reshape(-1)
  sel     = bernoulli(key 42, min(750/(count+1),1)[labels])   (host, jax CPU)
  nv      = feats / ||feats||
  dp      = exp(nv @ nv.T / TEMP), diagonal zeroed
  pos_i   = sum_{j sel, same class} dp_ij ; neg over different class
  loss    = mean over selected i of -log(pos/(pos+neg))

Only selected points contribute, so the device works on the compacted
point set (~37% of M).  Columns are sharded over 8 cores; each core
receives the compacted feature matrix *rolled* so its own columns sit
first, which pins the sim diagonal at a fixed (compile-time) position
for every core (SPMD-safe).

Per core (L = M_pad/8 local columns, nT = M_pad/128 row chunks,
grouped GS=3 chunks per PSUM tile):
  mm1 (PE):   G_t = nvT[:, chunk t].T @ nvT[:, :L]  fp16 -> [128, L] fp32,
              written into 512-aligned slots of a [128, 3*512] psum tile
  exp (ACT):  one wide strided ACTIVATE per group: dp = exp(G/TEMP),
              [128, 3, L] psum -> dense [128, 3*L] bf16 SBUF
              (the (N+352)cyc ACTIVATE overhead amortizes 3x vs per-chunk)
  diag (DVE): affine_select zeroes the 3 diagonal 128-blocks (group 0)
  mm2 (PE):   S += W_t.T @ dp_t  (W = sel*onehot(label) bf16, [4, L] psum)
The host gathers the per-core S blocks and finishes the O(n_sel) epilogue.
"""

import numpy as np

TEMP = 0.07
NUM_CLASSES = 4
N_CORES = 8
P = 128
GS = 2            # row chunks per psum tile / per ACTIVATE
SLOT = 512        # psum slot pitch (fp32 bank = 512 cols)

_NEFF_CACHE = {}
_results = [None]


def _compute_sel(labels_flat):
    """Selection mask, bit-exact with the reference (jax threefry, key 42)."""
    import jax
    import jax.numpy as jnp

    cpu = jax.devices("cpu")[0]
    with jax.default_device(cpu):
        lab_j = jnp.asarray(labels_flat)
        counts = jnp.bincount(lab_j, length=NUM_CLASSES)
        keep_p = jnp.minimum(750.0 / (counts.astype(jnp.float32) + 1.0), 1.0)
        p = keep_p[lab_j]
        sel = jax.random.bernoulli(jax.random.key(42), p)
        return np.asarray(sel)


def _build_kernel(M_pad):
    import concourse.bass as bass
    import concourse.mybir as mybir
    import concourse.tile as tile

    L = M_pad // N_CORES          # local columns per core
    nT = M_pad // P               # 128-row chunks
    nL = L // P                   # chunks containing this core's diagonal
    f32 = mybir.dt.float32
    f16 = mybir.dt.float16
    bf16 = mybir.dt.bfloat16

    nc = bass.Bass()
    nvt_d = nc.dram_tensor("nvt", [64, M_pad], f16, kind="ExternalInput")
    w_d = nc.dram_tensor("w", [P, nT * NUM_CLASSES], bf16, kind="ExternalInput")
    s_d = nc.dram_tensor("s_out", [NUM_CLASSES, L], f32, kind="ExternalOutput")

    groups = [(g, min(GS, nT - g)) for g in range(0, nT, GS)]
    # DMA fan-in: rhs + earliest lhsT chunks first (own queue), rest split
    # across otherwise-idle engine queues.
    cut1 = min(4 * P, M_pad)
    cut2 = min((nT // 2) * P, M_pad)

    with tile.TileContext(nc) as tc:
        with (
            tc.tile_pool(name="singles", bufs=1) as singles,
            tc.tile_pool(name="dp", bufs=3) as dp_pool,
            tc.tile_pool(name="ps", bufs=4, space="PSUM") as ps_pool,
        ):
            nvt = singles.tile([64, M_pad], f16)
            w_sb = singles.tile([P, nT * NUM_CLASSES], bf16)
            nc.gpsimd.dma_start(out=nvt[:, 0:cut1], in_=nvt_d[:, 0:cut1])
            if cut1 < cut2:
                nc.sync.dma_start(out=nvt[:, cut1:cut2], in_=nvt_d[:, cut1:cut2])
            if cut2 < M_pad:
                nc.scalar.dma_start(out=nvt[:, cut2:M_pad], in_=nvt_d[:, cut2:M_pad])
            nc.gpsimd.dma_start(out=w_sb[:], in_=w_d[:])

            s_sb = singles.tile([NUM_CLASSES, L], f32)
            nc.gpsimd.memset(s_sb[:], 0.0)

            rhs = nvt[:, 0:L]

            def mm2_and_acc(ps, dp, g0, n):
                # class sums for this group's chunks: partials go into this
                # group's (already-consumed) psum tile, then DVE folds them
                # into the SBUF accumulator — no dedicated psum bank needed.
                for i in range(n):
                    t = g0 + i
                    nc.tensor.matmul(
                        ps[0:NUM_CLASSES, i * SLOT:i * SLOT + L],
                        w_sb[:, t * NUM_CLASSES:(t + 1) * NUM_CLASSES],
                        dp[:, i * L:(i + 1) * L],
                        start=True, stop=True,
                    )
                for i in range(n):
                    nc.vector.tensor_add(
                        s_sb[:], s_sb[:], ps[0:NUM_CLASSES, i * SLOT:i * SLOT + L]
                    )

            prev = None          # deferred mm2 work: (ps, dp, g0, n)
            for g0, n in groups:
                ps = ps_pool.tile([P, GS * SLOT], f32)
                for i in range(n):
                    t = g0 + i
                    nc.tensor.matmul(
                        ps[:, i * SLOT:i * SLOT + L],
                        nvt[:, t * P:(t + 1) * P], rhs,
                        start=True, stop=True,
                    )
                dp = dp_pool.tile([P, GS * L], bf16)
                src = ps[:, 0:n * SLOT].rearrange("p (g c) -> p g c", c=SLOT)
                src = src[:, :, 0:L]
                dst = dp[:, 0:n * L].rearrange("p (g c) -> p g c", c=L)
                nc.scalar.activation(
                    dst, src, mybir.ActivationFunctionType.Exp,
                    scale=float(1.0 / TEMP),
                )
                for i in range(n):
                    # zero the diagonal: chunk t's diagonal element sits at
                    # dp[p, i*L + t*P + p]; keep where (col - p) != 0.
                    t = g0 + i
                    if t >= nL:
                        break
                    blk = dp[:, i * L + t * P: i * L + t * P + P]
                    nc.gpsimd.affine_select(
                        out=blk, in_=blk,
                        compare_op=mybir.AluOpType.not_equal,
                        fill=0.0, base=0,
                        pattern=[[1, P]], channel_multiplier=-1,
                    )
                if prev is not None:
                    mm2_and_acc(*prev)
                prev = (ps, dp, g0, n)
            mm2_and_acc(*prev)

            nc.gpsimd.dma_start(out=s_d[:], in_=s_sb[:])

    _split_multi_waits(nc)
    return nc


def _split_multi_waits(nc):
    """Walrus in this toolchain accepts only one inline sync-wait per
    instruction.  Tile's kernel-tail drain aggregates one wait per live
    semaphore, so hoist all but the last wait onto same-engine nops."""
    import concourse.mybir as mybir

    for fn in nc.m.functions:
        for blk in fn.blocks:
            insts = list(blk.instructions)
            out = []
            for inst in insts:
                si = inst.sync_info
                waits = list(si.on_wait) if si is not None and si.on_wait else []
                if len(waits) > 1:
                    for w in waits[:-1]:
                        out.append(mybir.InstNoOp(
                            name=nc.get_next_instruction_name(),
                            engine=inst.engine,
                            bass_nofuse=True,
                            sync_info=mybir.SyncInfo(on_wait=[w], on_update=[]),
                        ))
                    si.on_wait = waits[-1:]
                out.append(inst)
            if len(out) != len(insts):
                blk.instructions = out


def _get_kernel(M_pad):
    if M_pad not in _NEFF_CACHE:
        _NEFF_CACHE[M_pad] = _build_kernel(M_pad)
    return _NEFF_CACHE[M_pad]


def kernel(features_in, labels_in, _trace=False, _results=_results):
    import ml_dtypes
    from concourse.bass_utils import run_bass_kernel_spmd

    features_in = np.asarray(features_in, dtype=np.float32)
    B, C, N = features_in.shape
    M = B * N
    labels = np.asarray(labels_in).reshape(-1).astype(np.int64)

    fT = features_in.reshape(C, M)                      # [C, M] reinterpret
    sel = _compute_sel(labels)
    idx = np.nonzero(sel)[0]
    n_sel = int(idx.size)
    n_div = max(n_sel, 1)

    norms = np.sqrt(np.sum(fT * fT, axis=0, dtype=np.float32)).astype(np.float32)
    nvT = (fT / norms).astype(np.float32)

    lab_sel = labels[idx]
    per_core = N_CORES * P
    M_pad = max(((n_sel + per_core - 1) // per_core) * per_core, per_core)
    L = M_pad // N_CORES
    nT = M_pad // P

    nvT_pad = np.zeros((C, M_pad), np.float16)
    nvT_pad[:, :n_sel] = nvT[:, idx].astype(np.float16)
    W = np.zeros((M_pad, NUM_CLASSES), np.float32)
    W[np.arange(n_sel), lab_sel] = 1.0

    in_maps = []
    for k in range(N_CORES):
        nv_k = np.ascontiguousarray(np.roll(nvT_pad, -L * k, axis=1))
        W_k = np.roll(W, -L * k, axis=0)
        # lhsT chunk t lives at columns [4t, 4t+4): w_arr[p, 4t+c] = W_k[128t+p, c]
        w_arr = W_k.reshape(nT, P, NUM_CLASSES).transpose(1, 0, 2).reshape(
            P, nT * NUM_CLASSES
        ).astype(ml_dtypes.bfloat16)
        in_maps.append({"nvt": nv_k, "w": w_arr})

    nc = _get_kernel(M_pad)
    res = run_bass_kernel_spmd(nc, in_maps, core_ids=list(range(N_CORES)),
                               trace=_trace)
    _results[0] = res

    S = np.concatenate([res.results[k]["s_out"] for k in range(N_CORES)], axis=1)
    S = S[:, :n_sel].astype(np.float64)
    denom = np.sum(S, axis=0)
    numer = S[lab_sel, np.arange(n_sel)]
    per = -np.log(numer / denom)
    loss = np.float32(per.sum() / n_div)
    return np.asarray(loss, dtype=np.float32)


# revision 19
# speedup vs baseline: 1.9713x; 1.0082x over previous
"""Trainium2 Bass kernel for the intra-batch point-cloud contrastive loss.

Math (matches the reference exactly):
  feats   = features_in.reshape(C, M).T    (row-major reinterpret), M = B*N
  labels  = labels_in.reshape(-1)
  sel     = bernoulli(key 42, min(750/(count+1),1)[labels])   (host, jax CPU)
  nv      = feats / ||feats||
  dp      = exp(nv @ nv.T / TEMP), diagonal zeroed
  pos_i   = sum_{j sel, same class} dp_ij ; neg over different class
  loss    = mean over selected i of -log(pos/(pos+neg))

Only selected points contribute, so the device works on the compacted
point set (~37% of M).  Columns are sharded over 8 cores; each core
receives the compacted feature matrix *rolled* so its own columns sit
first, which pins the sim diagonal at a fixed (compile-time) position
for every core (SPMD-safe).

Per core (L = M_pad/8 local columns, nT = M_pad/128 row chunks,
grouped GS=3 chunks per PSUM tile):
  mm1 (PE):   G_t = nvT[:, chunk t].T @ nvT[:, :L]  fp16 -> [128, L] fp32,
              written into 512-aligned slots of a [128, 3*512] psum tile
  exp (ACT):  one wide strided ACTIVATE per group: dp = exp(G/TEMP),
              [128, 3, L] psum -> dense [128, 3*L] bf16 SBUF
              (the (N+352)cyc ACTIVATE overhead amortizes 3x vs per-chunk)
  diag (DVE): affine_select zeroes the 3 diagonal 128-blocks (group 0)
  mm2 (PE):   S += W_t.T @ dp_t  (W = sel*onehot(label) bf16, [4, L] psum)
The host gathers the per-core S blocks and finishes the O(n_sel) epilogue.
"""

import numpy as np

TEMP = 0.07
NUM_CLASSES = 4
N_CORES = 8
P = 128
GS = 2            # row chunks per psum tile / per ACTIVATE
SLOT = 512        # psum slot pitch (fp32 bank = 512 cols)

_NEFF_CACHE = {}
_results = [None]


def _compute_sel(labels_flat):
    """Selection mask, bit-exact with the reference (jax threefry, key 42)."""
    import jax
    import jax.numpy as jnp

    cpu = jax.devices("cpu")[0]
    with jax.default_device(cpu):
        lab_j = jnp.asarray(labels_flat)
        counts = jnp.bincount(lab_j, length=NUM_CLASSES)
        keep_p = jnp.minimum(750.0 / (counts.astype(jnp.float32) + 1.0), 1.0)
        p = keep_p[lab_j]
        sel = jax.random.bernoulli(jax.random.key(42), p)
        return np.asarray(sel)


def _build_kernel(M_pad):
    import concourse.bass as bass
    import concourse.mybir as mybir
    import concourse.tile as tile

    L = M_pad // N_CORES          # local columns per core
    nT = M_pad // P               # 128-row chunks
    nL = L // P                   # chunks containing this core's diagonal
    f32 = mybir.dt.float32
    f16 = mybir.dt.float16
    bf16 = mybir.dt.bfloat16

    nc = bass.Bass()
    nvt_d = nc.dram_tensor("nvt", [64, M_pad], f16, kind="ExternalInput")
    w_d = nc.dram_tensor("w", [P, nT * NUM_CLASSES], bf16, kind="ExternalInput")
    s_d = nc.dram_tensor("s_out", [36, L], f32, kind="ExternalOutput")

    NSLOT = 8                     # psum slots (one bank each)
    GW = 4                        # chunks per ACTIVATE group
    NWARM = 12                    # PE warm-up matmuls (HAM clock ramp)
    groups = [(g, min(GW, nT - g)) for g in range(0, nT, GW)]
    # DMA fan-in: rhs + earliest lhsT chunks on the idle Sync queue (it
    # triggers earliest), the rest split across the other DGE queues.
    cut1 = min(4 * P, M_pad)
    cut2 = min((nT // 2) * P, M_pad)

    with tile.TileContext(nc) as tc:
        with (
            tc.tile_pool(name="singles", bufs=1) as singles,
            tc.tile_pool(name="dp", bufs=3) as dp_pool,
            tc.tile_pool(name="ps", bufs=1, space="PSUM") as ps_pool,
        ):
            nvt = singles.tile([64, M_pad], f16)
            w_sb = singles.tile([P, nT * NUM_CLASSES], bf16)
            warm = singles.tile([64, 4 * P], f16)   # scratch for PE warm-up
            nc.gpsimd.memset(warm[:], 0.0)
            nc.sync.dma_start(out=nvt[:, 0:cut1], in_=nvt_d[:, 0:cut1])
            if cut1 < cut2:
                nc.scalar.dma_start(out=nvt[:, cut1:cut2], in_=nvt_d[:, cut1:cut2])
            if cut2 < M_pad:
                nc.gpsimd.dma_start(out=nvt[:, cut2:M_pad], in_=nvt_d[:, cut2:M_pad])
            nc.gpsimd.dma_start(out=w_sb[:], in_=w_d[:])

            s_sb = singles.tile([36, L], f32)
            nc.gpsimd.memset(s_sb[:], 0.0)

            ps = ps_pool.tile([P, NSLOT * SLOT], f32)
            # Keep the PE continuously busy from program start so the HAM
            # clock gate promotes it to full rate before real data lands.
            # Garbage operands; results land in slot 7 and are overwritten.
            for _ in range(NWARM):
                nc.tensor.matmul(
                    ps[:, 7 * SLOT:7 * SLOT + L],
                    warm[:, 0:P], warm[:, P:P + L],
                    start=True, stop=True,
                )

            rhs = nvt[:, 0:L]

            def mm2_and_acc(dp, g0, n):
                # class sums for this group's chunks: stack two chunks per psum
                # slot (partition bases 0 and 32 — PE requires {0,32,64}), then
                # one DVE add per slot folds rows [0:36] into SBUF; rows 4..31
                # carry finite mm1 leftovers the host never reads.
                reg = (g0 % NSLOT) * SLOT
                for i in range(n):
                    t = g0 + i
                    sub = reg + (i // 2) * SLOT
                    bp = (i % 2) * 32
                    nc.tensor.matmul(
                        ps[bp:bp + NUM_CLASSES, sub:sub + L],
                        w_sb[:, t * NUM_CLASSES:(t + 1) * NUM_CLASSES],
                        dp[:, i * L:(i + 1) * L],
                        start=True, stop=True,
                    )
                for h in range((n + 1) // 2):
                    nc.vector.tensor_add(
                        s_sb[:], s_sb[:],
                        ps[0:36, reg + h * SLOT:reg + h * SLOT + L],
                    )

            prev = None          # deferred mm2 work: (dp, g0, n)
            for g0, n in groups:
                base = (g0 % NSLOT) * SLOT
                for i in range(n):
                    t = g0 + i
                    nc.tensor.matmul(
                        ps[:, base + i * SLOT:base + i * SLOT + L],
                        nvt[:, t * P:(t + 1) * P], rhs,
                        start=True, stop=True,
                    )
                dp = dp_pool.tile([P, GW * L], bf16)
                src = ps[:, base:base + n * SLOT].rearrange(
                    "p (g c) -> p g c", c=SLOT
                )[:, :, 0:L]
                dst = dp[:, 0:n * L].rearrange("p (g c) -> p g c", c=L)
                nc.scalar.activation(
                    dst, src, mybir.ActivationFunctionType.Exp,
                    scale=float(1.0 / TEMP),
                )
                for i in range(n):
                    # zero the diagonal: chunk t's diagonal element sits at
                    # dp[p, i*L + t*P + p]; keep where (col - p) != 0.
                    t = g0 + i
                    if t >= nL:
                        break
                    blk = dp[:, i * L + t * P: i * L + t * P + P]
                    nc.gpsimd.affine_select(
                        out=blk, in_=blk,
                        compare_op=mybir.AluOpType.not_equal,
                        fill=0.0, base=0,
                        pattern=[[1, P]], channel_multiplier=-1,
                    )
                if prev is not None:
                    mm2_and_acc(*prev)
                prev = (dp, g0, n)
            mm2_and_acc(*prev)

            nc.sync.dma_start(out=s_d[:], in_=s_sb[:])

    _split_multi_waits(nc)
    return nc


def _split_multi_waits(nc):
    """Walrus in this toolchain accepts only one inline sync-wait per
    instruction.  Tile's kernel-tail drain aggregates one wait per live
    semaphore, so hoist all but the last wait onto same-engine nops."""
    import concourse.mybir as mybir

    for fn in nc.m.functions:
        for blk in fn.blocks:
            insts = list(blk.instructions)
            out = []
            for inst in insts:
                si = inst.sync_info
                waits = list(si.on_wait) if si is not None and si.on_wait else []
                if len(waits) > 1:
                    for w in waits[:-1]:
                        out.append(mybir.InstNoOp(
                            name=nc.get_next_instruction_name(),
                            engine=inst.engine,
                            bass_nofuse=True,
                            sync_info=mybir.SyncInfo(on_wait=[w], on_update=[]),
                        ))
                    si.on_wait = waits[-1:]
                out.append(inst)
            if len(out) != len(insts):
                blk.instructions = out


def _get_kernel(M_pad):
    if M_pad not in _NEFF_CACHE:
        _NEFF_CACHE[M_pad] = _build_kernel(M_pad)
    return _NEFF_CACHE[M_pad]


def kernel(features_in, labels_in, _trace=False, _results=_results):
    import ml_dtypes
    from concourse.bass_utils import run_bass_kernel_spmd

    features_in = np.asarray(features_in, dtype=np.float32)
    B, C, N = features_in.shape
    M = B * N
    labels = np.asarray(labels_in).reshape(-1).astype(np.int64)

    fT = features_in.reshape(C, M)                      # [C, M] reinterpret
    sel = _compute_sel(labels)
    idx = np.nonzero(sel)[0]
    n_sel = int(idx.size)
    n_div = max(n_sel, 1)

    norms = np.sqrt(np.sum(fT * fT, axis=0, dtype=np.float32)).astype(np.float32)
    nvT = (fT / norms).astype(np.float32)

    lab_sel = labels[idx]
    per_core = N_CORES * P
    M_pad = max(((n_sel + per_core - 1) // per_core) * per_core, per_core)
    L = M_pad // N_CORES
    nT = M_pad // P

    nvT_pad = np.zeros((C, M_pad), np.float16)
    nvT_pad[:, :n_sel] = nvT[:, idx].astype(np.float16)
    W = np.zeros((M_pad, NUM_CLASSES), np.float32)
    W[np.arange(n_sel), lab_sel] = 1.0

    in_maps = []
    for k in range(N_CORES):
        nv_k = np.ascontiguousarray(np.roll(nvT_pad, -L * k, axis=1))
        W_k = np.roll(W, -L * k, axis=0)
        # lhsT chunk t lives at columns [4t, 4t+4): w_arr[p, 4t+c] = W_k[128t+p, c]
        w_arr = W_k.reshape(nT, P, NUM_CLASSES).transpose(1, 0, 2).reshape(
            P, nT * NUM_CLASSES
        ).astype(ml_dtypes.bfloat16)
        in_maps.append({"nvt": nv_k, "w": w_arr})

    nc = _get_kernel(M_pad)
    res = run_bass_kernel_spmd(nc, in_maps, core_ids=list(range(N_CORES)),
                               trace=_trace)
    _results[0] = res

    S = np.concatenate(
        [res.results[k]["s_out"][0:4] + res.results[k]["s_out"][32:36]
         for k in range(N_CORES)], axis=1)
    S = S[:, :n_sel].astype(np.float64)
    denom = np.sum(S, axis=0)
    numer = S[lab_sel, np.arange(n_sel)]
    per = -np.log(numer / denom)
    loss = np.float32(per.sum() / n_div)
    return np.asarray(loss, dtype=np.float32)
